# revision 1
# baseline (speedup 1.0000x reference)
"""Trainium2 Bass kernel for DariushFlashAttention2 (per-token [H,H] head
attention) — v2 layout-optimized design.

Math per token: Q,K,V rows reshape to [H=32, D=128];
  L = Q K^T / sqrt(D) (32x32), W = softmax(L, axis=-1), O = W V.
8192 tokens are independent -> 1024 tokens/core across 8 cores, processed
in 16 chunks of 64 tokens.

v2-quad key ideas (vs the v1 checkpoint at 202us cost-model / 562us graded
/ 277us slope-timed; v2-base measured 159us slope-timed on HW):
- HOST-side prep: inputs cast to bf16 AND pre-transposed so every DMA is
  fully contiguous (4KB per partition line). Halves DMA bytes at full
  360GB/s aggregate -> DMA ~93us instead of 186us.
    qt/kt dram [chunk, d=128, 32*r+h]   (r = token-in-chunk 0..63)
    v   dram [chunk, 32*a+g, 128*m+d]   (r = 16a+m)
    out dram [chunk, d=128, 32*r+h]     (bf16; host upcasts/reorders)
- mm1 full-K: per token ONE matmul K=128 (lhsT=Q^T col-slice, rhs=K^T
  col-slice), M=32, N=32, tile_position=(0,32a) stacks 4 a-groups onto
  128 PSUM partitions; whole chunk's logits fill ONE PSUM bank [128,512]
  via one start/stop accumulation group per 32-partition row (start only
  zeroes the 2KB zero-region once; disjoint col writes accumulate onto
  pending-zero bytes). 4x fewer PE instructions and 4x fewer moving rows
  than v1's K=32 quadrant scheme.
- exp: ONE ScalarE activation per chunk [128,512] PSUM->SBUF bf16 with
  scale=1/sqrt(D); no accum_out (the accumulator read costs +187ns/instr).
  Z comes from a single DVE segmented reduce ([128,16,32] -> [128,16]).
- softmax normalize on E (32 cols/token) instead of O (128 cols/token):
  reciprocal + 16 tensor_scalar muls, then one DVE 32x32 block-transpose
  [128,512] -> eT.
- mm2 QUAD-BATCHED (variant 'quad'): LDWEIGHTS on HW costs ~M/1.2 ns
  (M = stationary columns; FWL halves it only when M=128, non-fp32), so
  per-token mm2 pays a disproportionate weight-load bill. Instead build a
  block-diagonal W~ [128,128] per quad m (4 tokens stacked on the
  contraction axis): out[d, 32t+h] = sum_{t,g} V_quad[32t+g, d]
  W~[32t+g, 32t+h] -> ONE K=128/M=128(FWL)/N=128 matmul per 4 tokens
  (~81ns/quad HW roofline). W~ tiles are persistent (zeros memset once,
  only the 16x4 diagonal 32x32 blocks are rewritten each chunk via
  strided DVE/Pool copies from eT).
- input DMAs on the SP HWDGE queue, output DMAs on the ACT HWDGE queue
  (avoids head-of-line blocking of chunk n+1 inputs behind chunk n's
  output-DMA semaphore wait). PSUM->SBUF epilogue drains alternate
  ACT/DVE (gpsimd cannot touch PSUM).

Cost-model projection ~102us (DMA 91% busy); the quad-mm2 win is on HW
(cost model doesn't price LDWEIGHTS).
"""

import math

import numpy as np

NUM_CORES = 8
B, S, E = 2, 4096, 4096
H, D = 32, 128
T_TOTAL = B * S  # 8192 tokens
T_CORE = T_TOTAL // NUM_CORES  # 1024
CHUNK_TOKENS = 64
N_CHUNKS = T_CORE // CHUNK_TOKENS  # 16
INV_SQRT_D = 1.0 / math.sqrt(D)
VARIANT = "quad"

_CACHE = {}


def _null_ctx():
    import contextlib

    return contextlib.nullcontext()


def _build_bass(n_chunks=N_CHUNKS, loop_reps=None, variant="base"):
    import concourse.bacc as bacc
    import concourse.tile as tile
    from concourse import mybir

    fp32 = mybir.dt.float32
    bf16 = mybir.dt.bfloat16
    Exp = mybir.ActivationFunctionType.Exp
    Copy = mybir.ActivationFunctionType.Copy

    nc = bacc.Bacc()

    # q/k/v packed side by side -> ONE input DMA per chunk (the per-DMA
    # HWDGE overhead is ~625ns fixed, comparable to a 512KB transfer).
    qkv = nc.dram_tensor("qkv", [n_chunks, 128, 6144], bf16, kind="ExternalInput")
    out = nc.dram_tensor("out", [n_chunks, 128, 2048], bf16, kind="ExternalOutput")

    big_bufs = 3
    if variant.startswith("bufs"):
        big_bufs = int(variant[4:])
    elif variant == "pipe":
        big_bufs = 4
    elif variant == "quad":
        big_bufs = 5
    with tile.TileContext(nc) as tc:
        with (
            tc.tile_pool(name="big", bufs=big_bufs) as big,
            tc.tile_pool(name="small", bufs=3) as small,
            tc.tile_pool(name="psum_l", bufs=2, space="PSUM") as psum_l,
            tc.tile_pool(name="psum_o", bufs=6, space="PSUM") as psum_o,
            tc.tile_pool(name="bdp", bufs=1) as bdp,
            tc.For_i(0, loop_reps, 1) if loop_reps else _null_ctx(),
        ):
            dmaonly = variant == "dmaonly"
            pipe = variant == "pipe"
            quad = variant in ("quad", "nodma", "sp")
            nodma = variant == "nodma"
            skew = variant == "sp"
            halves = ((0, 8), (8, 16)) if pipe else ((0, 16),)
            bds = []
            if quad:
                # Persistent block-diagonal W~ tiles (ping-pong by chunk
                # parity). Zeros written once; only diagonal blocks are
                # rewritten each chunk.
                for i in range(4):
                    bd = bdp.tile([128, 2048], bf16, name=f"bd{i}", tag=f"bd{i}")
                    nc.gpsimd.memset(bd[:, :], 0.0)
                    bds.append(bd)
            qkvb_shared = None
            if nodma:
                qkvb_shared = big.tile([128, 6144], bf16, tag="qkvb1", name="qkvb1")
                nc.sync.dma_start(out=qkvb_shared, in_=qkv[0])
            for ch in range(n_chunks):
                if nodma:
                    qkvb = qkvb_shared
                else:
                    qkvb = big.tile([128, 6144], bf16, tag="qkvb", name="qkvb")
                    nc.sync.dma_start(out=qkvb, in_=qkv[ch])
                if dmaonly:
                    nc.scalar.dma_start(out=out[ch], in_=qkvb[:, 0:2048])
                    continue

                pl = psum_l.tile([128, 512], fp32, tag="pl", name="pl")
                e = small.tile([128, 512], bf16, tag="e", name="e")
                z = small.tile([128, 16], fp32, tag="z", name="z")
                rz = small.tile([128, 16], fp32, tag="rz", name="rz")
                es = small.tile([128, 512], bf16, tag="es", name="es")
                et = small.tile([128, 512], bf16, tag="et", name="et")
                outc = big.tile([128, 2048], bf16, tag="outc", name="outc")
                pos = [
                    psum_o.tile([128, 512], fp32, tag="po", name="po")
                    for _ in range(4)
                ]

                for m0, m1 in halves:
                    nm = m1 - m0
                    # mm1: L[h,g] per token r=16a+m -> pl[32a+h, 32m+g].
                    # One accumulation group per 32-partition row: start
                    # zeroes the 2KB zero-region once; later disjoint-col
                    # matmuls accumulate onto pending-zero bytes (= write).
                    for a in range(4):
                        for m in range(m0, m1):
                            r = 16 * a + m
                            nc.tensor.matmul(
                                pl[32 * a : 32 * a + 32, 32 * m : 32 * m + 32],
                                lhsT=qkvb[:, 32 * r : 32 * r + 32],
                                rhs=qkvb[:, 2048 + 32 * r : 2048 + 32 * r + 32],
                                start=(m == m0),
                                stop=(m == m1 - 1),
                                tile_position=(0, 32 * a),
                            )

                    cs = slice(32 * m0, 32 * m1)
                    ms = slice(m0, m1)
                    # E = exp(L/sqrt(D)) in one ACT instr; Z via segmented
                    # DVE reduce (no ACT accum_out: +187ns/instr); normalize
                    # E (32 cols/token, 4x cheaper than scaling O); 32x32
                    # block-transpose -> eT[32a+g, 32m+h].
                    nc.scalar.activation(
                        out=e[:, cs], in_=pl[:, cs], func=Exp, scale=INV_SQRT_D
                    )
                    nc.vector.tensor_reduce(
                        z[:, ms],
                        e[:, cs].rearrange("p (m g) -> p m g", g=32),
                        axis=mybir.AxisListType.X,
                        op=mybir.AluOpType.add,
                    )
                    nc.vector.reciprocal(rz[:, ms], z[:, ms])
                    nc.vector.tensor_tensor(
                        out=es[:, cs].rearrange("p (m g) -> p m g", g=32),
                        in0=e[:, cs].rearrange("p (m g) -> p m g", g=32),
                        in1=rz[:, ms].broadcast_to([128, nm, 32]),
                        op=mybir.AluOpType.mult,
                    )
                    nc.vector.transpose(et[:, cs], es[:, cs])

                    if quad:
                        # Scatter eT blocks onto the diagonal of the
                        # persistent W~ tile: bd[32a+g, 128m+32a+h].
                        # (gpsimd may not touch PSUM, but these are SBUF->SBUF
                        # so spread them across DVE + Pool.)
                        bd = bds[ch % 4]
                        for a in range(4):
                            src = et[32 * a : 32 * a + 32, :].rearrange(
                                "p (m h) -> p m h", h=32
                            )
                            dst = bd[32 * a : 32 * a + 32, :].rearrange(
                                "p (m x) -> p m x", x=128
                            )[:, :, 32 * a : 32 * a + 32]
                            if a % 2 == 0:
                                nc.vector.tensor_copy(dst, src)
                            else:
                                nc.gpsimd.tensor_copy(dst, src)
                        # mm2 quad-batched: ONE matmul per 4 tokens (quad m):
                        # out[d, 32t+h] = sum_{t,g} V[32t+g, d] W~[32t+g, 32t+h]
                        # K=128, M=128 (FWL), N=128 -> ~81ns/quad on HW.
                        for m in range(16):
                            j, c = m // 4, m % 4
                            nc.tensor.matmul(
                                pos[j][:, 128 * c : 128 * c + 128],
                                lhsT=qkvb[:, 4096 + 128 * m : 4096 + 128 * m + 128],
                                rhs=bd[:, 128 * m : 128 * m + 128],
                                start=(c == 0),
                                stop=(c == 3),
                            )
                            if c == 3:
                                # PSUM->SBUF drain: alternate ACT / DVE
                                dst = outc[:, 512 * j : 512 * j + 512]
                                if j % 2 == 0:
                                    nc.scalar.activation(
                                        out=dst, in_=pos[j], func=Copy
                                    )
                                else:
                                    nc.vector.tensor_copy(dst, pos[j])
                        continue

                    # mm2 transposed: O^T[d,h] = sum_g V[g,d] W[h,g].
                    # Bank a holds tokens r=16a+m (one group per bank).
                    for a in range(4):
                        for m in range(m0, m1):
                            nc.tensor.matmul(
                                pos[a][:, 32 * m : 32 * m + 32],
                                lhsT=qkvb[
                                    32 * a : 32 * a + 32,
                                    4096 + 128 * m : 4096 + 128 * m + 128,
                                ],
                                rhs=et[32 * a : 32 * a + 32, 32 * m : 32 * m + 32],
                                start=(m == 0),
                                stop=(m == 15),
                                tile_position=(32 * a, 0),
                            )
                        if m1 == 16:
                            # bank complete: drain PSUM->SBUF (alternate
                            # ACT/DVE), out-DMA per bank pair for tail overlap
                            dst = outc[:, 512 * a : 512 * a + 512]
                            if a % 2 == 0:
                                nc.scalar.activation(out=dst, in_=pos[a], func=Copy)
                            else:
                                nc.vector.tensor_copy(dst, pos[a])
                                if pipe:
                                    nc.scalar.dma_start(
                                        out=out[ch][:, 1024 * (a // 2) :][
                                            :, 0:1024
                                        ],
                                        in_=outc[:, 1024 * (a // 2) :][:, 0:1024],
                                    )
                if not pipe and not nodma:
                    nc.scalar.dma_start(out=out[ch], in_=outc)

    nc.finalize()
    return nc


def get_nc(n_chunks=N_CHUNKS, loop_reps=None, variant="base"):
    key = ("nc", n_chunks, loop_reps, variant)
    if key not in _CACHE:
        _CACHE[key] = _build_bass(n_chunks, loop_reps, variant)
    return _CACHE[key]


def _np_bf16():
    from concourse import mybir

    return mybir.dt.np(mybir.dt.bfloat16)


def prep_inputs(q, k, v, n_cores=NUM_CORES, n_chunks=N_CHUNKS):
    """Full [T,E] fp32 -> per-core packed DMA-contiguous bf16 layout
    [core, chunk, 128, 6144] with q cols 0:2048 / k 2048:4096 / v 4096:6144."""
    bf = _np_bf16()
    t = n_cores * n_chunks * CHUNK_TOKENS
    qb = np.asarray(q, np.float32).reshape(t, E).astype(bf)
    kb = np.asarray(k, np.float32).reshape(t, E).astype(bf)
    vb = np.asarray(v, np.float32).reshape(t, E).astype(bf)
    qkv = np.empty((n_cores, n_chunks, 128, 6144), bf)
    # [core, chunk, r, h, d] -> [core, chunk, d, r, h]
    qkv[:, :, :, 0:2048] = (
        qb.reshape(n_cores, n_chunks, 64, 32, 128).transpose(0, 1, 4, 2, 3)
    ).reshape(n_cores, n_chunks, 128, 2048)
    qkv[:, :, :, 2048:4096] = (
        kb.reshape(n_cores, n_chunks, 64, 32, 128).transpose(0, 1, 4, 2, 3)
    ).reshape(n_cores, n_chunks, 128, 2048)
    # [core, chunk, a, m, g, d] -> [core, chunk, a, g, m, d]
    qkv[:, :, :, 4096:6144] = (
        vb.reshape(n_cores, n_chunks, 4, 16, 32, 128).transpose(0, 1, 2, 4, 3, 5)
    ).reshape(n_cores, n_chunks, 128, 2048)
    return qkv


def post_output(outs, n_chunks=N_CHUNKS, layout="quad"):
    """Per-core [n_chunks,128,2048] bf16 O^T tiles -> full [T,E] fp32.

    layout 'base': col = 32*r + h (r = token-in-chunk).
    layout 'quad': col = 128*m + 32*t + h (token r = 16*t + m).
    """
    o = np.stack([np.asarray(x) for x in outs])
    n_cores = o.shape[0]
    if layout == "quad":
        o = o.reshape(n_cores, n_chunks, 128, 16, 4, 32).transpose(0, 1, 4, 3, 5, 2)
    else:
        o = o.reshape(n_cores, n_chunks, 128, 64, 32).transpose(0, 1, 3, 4, 2)
    return (
        np.ascontiguousarray(o)
        .astype(np.float32)
        .reshape(n_cores * n_chunks * 64, E)
    )


def _build_warmup():
    """Tiny 8-core memcpy kernel used to shake out cold-device state before
    the first real execution (a fresh device has been observed to fail its
    very first heavy 8-core NEFF with EXEC_UNIT_UNRECOVERABLE)."""
    import concourse.bacc as bacc
    import concourse.tile as tile
    from concourse import mybir

    nc = bacc.Bacc()
    x = nc.dram_tensor("x", [128, 512], mybir.dt.float32, kind="ExternalInput")
    y = nc.dram_tensor("y", [128, 512], mybir.dt.float32, kind="ExternalOutput")
    with tile.TileContext(nc) as tc:
        with tc.tile_pool(name="p", bufs=1) as p:
            t = p.tile([128, 512], mybir.dt.float32, name="t")
            nc.sync.dma_start(out=t, in_=x[:, :])
            nc.sync.dma_start(out=y[:, :], in_=t)
    nc.finalize()
    return nc


def _warmup():
    from concourse.bass_utils import run_bass_kernel_spmd

    if "warm" in _CACHE:
        return
    nc = _build_warmup()
    x = np.zeros((128, 512), np.float32)
    try:
        run_bass_kernel_spmd(
            nc, [{"x": x} for _ in range(NUM_CORES)], core_ids=list(range(NUM_CORES))
        )
    except Exception:
        pass  # warmup is best-effort
    _CACHE["warm"] = True


def kernel(q, k, v, _trace=False):
    from concourse.bass_utils import run_bass_kernel_spmd

    qkv = prep_inputs(q, k, v)

    nc = get_nc(variant=VARIANT)
    in_maps = [{"qkv": qkv[c]} for c in range(NUM_CORES)]

    _warmup()
    res = None
    for attempt in range(3):
        try:
            res = run_bass_kernel_spmd(
                nc, in_maps, core_ids=list(range(NUM_CORES)), trace=_trace
            )
            break
        except Exception:
            if attempt == 2:
                raise
    full = post_output(
        [r["out"] for r in res.results],
        layout="quad" if VARIANT == "quad" else "base",
    ).reshape(B, S, E)
    if _trace:
        return full, res
    return full

